# revision 2
# baseline (speedup 1.0000x reference)
"""Causal multi-head attention on 8 trn2 NeuronCores.

Sharding: core c handles batch b=c//4 and heads [4*(c%4), 4*(c%4)+4).
Each core computes its 4 heads' attention plus the partial output
projection against the matching 256 rows of Wo; the host sums the 4
partials per batch (the all-reduce implied by row-sharding Wo).

Layout strategy (all fp32):
  - X^T [D,S] in SBUF so every matmul contracts d on partitions.
  - Q^T/K^T per head-pair [128, S] (two heads stacked on partitions),
    biases + 1/sqrt(dk) folded into the PSUM->SBUF eviction.
  - Scores computed transposed: ST[kv, q] = K^T.T @ Q^T. Causal mask is
    added in PSUM via an identity-weight matmul of a constant tile.
  - P = exp(ST) on ACT. Context ctxT[dk, q] = Vaug.T @ P where Vaug
    carries a ones column, so the softmax denominator lands in a spare
    PSUM partition of the same accumulation. V bias folds in post-norm
    because softmax rows sum to one.
  - Normalization: reciprocal of the denominator row, broadcast across
    partitions with a rank-1 PE matmul, one DVE multiply + bias add.
  - Output projection accumulates both 128-row chunks of ctx_cat^T plus
    a rank-1 bias matmul (bo only on cores with head-group 0).
"""

import sys

for _p in ("/opt/trn_rl_repo", "/root/.axon_site/_ro/trn_rl_repo"):
    if _p not in sys.path:
        sys.path.insert(0, _p)

import numpy as np

import concourse.bass as bass
import concourse.bacc as bacc
import concourse.tile as tile
from concourse import mybir
from concourse.bass_utils import run_bass_kernel_spmd

F32 = mybir.dt.float32
B, S, D, H, DK = 2, 2048, 1024, 16, 64
NCORES = 8
HPC = 4          # heads per core
NPAIR = 2        # head pairs per core
ND = D // 128    # 8 contraction chunks over d
NS = S // 512    # 4 query blocks
NS16 = S // 128  # 16 sequence chunks

_CACHE = {}


def _build_bass():
    nc = bacc.Bacc(None)
    xt = nc.dram_tensor("xt", [D, S], F32, kind="ExternalInput")
    wq = nc.dram_tensor("wq", [NPAIR, D, 128], F32, kind="ExternalInput")
    wk = nc.dram_tensor("wk", [NPAIR, D, 128], F32, kind="ExternalInput")
    wv = nc.dram_tensor("wv", [D, 256], F32, kind="ExternalInput")
    wo = nc.dram_tensor("wo", [256, D], F32, kind="ExternalInput")
    bq = nc.dram_tensor("bq", [128, NPAIR], F32, kind="ExternalInput")
    bk = nc.dram_tensor("bk", [128, NPAIR], F32, kind="ExternalInput")
    bv = nc.dram_tensor("bv", [128, HPC], F32, kind="ExternalInput")
    bo = nc.dram_tensor("bo", [1, D], F32, kind="ExternalInput")
    mneg = nc.dram_tensor("mneg", [128, 4, 512], F32, kind="ExternalInput")
    ident = nc.dram_tensor("ident", [128, 128], F32, kind="ExternalInput")
    ones = nc.dram_tensor("ones", [1, 128], F32, kind="ExternalInput")
    oneshalf = nc.dram_tensor("oneshalf", [1, 128], F32, kind="ExternalInput")
    out = nc.dram_tensor("out", [S, D], F32, kind="ExternalOutput")

    with tile.TileContext(nc) as tc:
        with (
            tc.tile_pool(name="consts", bufs=1) as consts,
            tc.tile_pool(name="qkv", bufs=1) as qkv,
            tc.tile_pool(name="mmp", bufs=2, space="PSUM") as mmp,
        ):
            wq_sb = consts.tile([128, NPAIR, ND, 128], F32, tag="wq")
            wk_sb = consts.tile([128, NPAIR, ND, 128], F32, tag="wk")
            wv_sb = consts.tile([128, ND, 256], F32, tag="wv")
            wo_sb = consts.tile([128, 2, D], F32, tag="wo")
            bq_sb = consts.tile([128, NPAIR], F32, tag="bq")
            bk_sb = consts.tile([128, NPAIR], F32, tag="bk")
            bv_sb = consts.tile([128, HPC], F32, tag="bv")
            bo_sb = consts.tile([1, D], F32, tag="bo")
            mneg_sb = consts.tile([128, 4, 512], F32, tag="mneg")
            ident_sb = consts.tile([128, 128], F32, tag="ident")
            ones_sb = consts.tile([1, 128], F32, tag="ones")
            oneshalf_sb = consts.tile([1, 128], F32, tag="oneshalf")

            for p in range(NPAIR):
                for c in range(ND):
                    nc.sync.dma_start(
                        out=wq_sb[:, p, c, :], in_=wq[p, c * 128:(c + 1) * 128, :]
                    )
                    nc.sync.dma_start(
                        out=wk_sb[:, p, c, :], in_=wk[p, c * 128:(c + 1) * 128, :]
                    )
            for c in range(ND):
                nc.sync.dma_start(
                    out=wv_sb[:, c, :], in_=wv[c * 128:(c + 1) * 128, :]
                )
            for k in range(2):
                nc.sync.dma_start(
                    out=wo_sb[:, k, :], in_=wo[k * 128:(k + 1) * 128, :]
                )
            nc.sync.dma_start(out=bq_sb[:], in_=bq[:])
            nc.sync.dma_start(out=bk_sb[:], in_=bk[:])
            nc.sync.dma_start(out=bv_sb[:], in_=bv[:])
            nc.sync.dma_start(out=bo_sb[:], in_=bo[:])
            nc.sync.dma_start(out=mneg_sb[:], in_=mneg[:])
            nc.sync.dma_start(out=ident_sb[:], in_=ident[:])
            nc.sync.dma_start(out=ones_sb[:], in_=ones[:])
            nc.sync.dma_start(out=oneshalf_sb[:], in_=oneshalf[:])

            # Persistent per-head-pair activations.
            qt_sb = qkv.tile([128, NPAIR, S], F32, tag="qt")
            kt_sb = qkv.tile([128, NPAIR, S], F32, tag="kt")
            # Vaug per pair: cols 0:64 V_even | 64 ones | 65:128 zeros
            # | 128:192 V_odd. Even lhsT = cols 0:65 -> ctx on parts
            # 0:64 (+denominator row 64); odd lhsT = cols 64:192 ->
            # denominator on part 0, ctx on parts 64:128.
            va_sb = qkv.tile([128, NPAIR, NS16, 192], F32, tag="va")
            ctxcat_sb = qkv.tile([128, 2, S], F32, tag="ctxcat")

            nc.vector.memset(va_sb[:], 0.0)
            for p in range(NPAIR):
                nc.vector.memset(va_sb[:, p, :, 64:65], 1.0)

            with tc.tile_pool(name="xp", bufs=1) as xp:
                xt_sb = xp.tile([128, ND, S], F32, tag="xt")
                for c in range(ND):
                    nc.sync.dma_start(
                        out=xt_sb[:, c, :], in_=xt[c * 128:(c + 1) * 128, :]
                    )

                # ---- Q^T / K^T projections (per pair, dk on partitions)
                for p in range(NPAIR):
                    for sb in range(NS):
                        qp = mmp.tile([128, 512], F32, tag="mm", name="qp")
                        for c in range(ND):
                            nc.tensor.matmul(
                                qp[:],
                                lhsT=wq_sb[:, p, c, :],
                                rhs=xt_sb[:, c, sb * 512:(sb + 1) * 512],
                                start=(c == 0),
                                stop=(c == ND - 1),
                            )
                        nc.scalar.activation(
                            out=qt_sb[:, p, sb * 512:(sb + 1) * 512],
                            in_=qp[:],
                            func=mybir.ActivationFunctionType.Identity,
                            bias=bq_sb[:, p:p + 1],
                            scale=0.125,
                        )
                        kp = mmp.tile([128, 512], F32, tag="mm", name="kp")
                        for c in range(ND):
                            nc.tensor.matmul(
                                kp[:],
                                lhsT=wk_sb[:, p, c, :],
                                rhs=xt_sb[:, c, sb * 512:(sb + 1) * 512],
                                start=(c == 0),
                                stop=(c == ND - 1),
                            )
                        nc.scalar.activation(
                            out=kt_sb[:, p, sb * 512:(sb + 1) * 512],
                            in_=kp[:],
                            func=mybir.ActivationFunctionType.Identity,
                            bias=bk_sb[:, p:p + 1],
                            scale=1.0,
                        )

                # ---- V in natural layout [s, dk], 4 heads at once
                for s16 in range(NS16):
                    vp = mmp.tile([128, 256], F32, tag="mm", name="vp")
                    for c in range(ND):
                        nc.tensor.matmul(
                            vp[:],
                            lhsT=xt_sb[:, c, s16 * 128:(s16 + 1) * 128],
                            rhs=wv_sb[:, c, :],
                            start=(c == 0),
                            stop=(c == ND - 1),
                        )
                    for h in range(HPC):
                        p, j = h // 2, h % 2
                        dst0 = 0 if j == 0 else 128
                        nc.vector.tensor_copy(
                            out=va_sb[:, p, s16, dst0:dst0 + 64],
                            in_=vp[:, h * 64:(h + 1) * 64],
                        )

            # ---- attention + output projection, per query block
            with (
                tc.tile_pool(name="stp", bufs=4, space="PSUM") as stp,
                tc.tile_pool(name="ctxp", bufs=2, space="PSUM") as ctxp,
                tc.tile_pool(name="ptp", bufs=6) as ptp,
                tc.tile_pool(name="smp", bufs=3) as smp,
                tc.tile_pool(name="outp", bufs=3) as outp,
            ):
                for qb in range(NS):
                    nch = (qb + 1) * 4
                    for h in range(HPC):
                        p, j = h // 2, h % 2
                        even = j == 0
                        qs = qt_sb[j * 64:(j + 1) * 64, p, qb * 512:(qb + 1) * 512]
                        ctx_ps = ctxp.tile([128, 512], F32, tag="ctx", name="ctx_ps")
                        ctx_out = ctx_ps[0:65, :] if even else ctx_ps[:]
                        for c in range(nch):
                            st = stp.tile([128, 512], F32, tag="st", name="st")
                            diag = c >= qb * 4
                            nc.tensor.matmul(
                                st[:],
                                lhsT=kt_sb[j * 64:(j + 1) * 64, p,
                                           c * 128:(c + 1) * 128],
                                rhs=qs,
                                start=True,
                                stop=not diag,
                            )
                            if diag:
                                nc.tensor.matmul(
                                    st[:],
                                    lhsT=ident_sb[:],
                                    rhs=mneg_sb[:, c - qb * 4, :],
                                    start=False,
                                    stop=True,
                                )
                            pt = ptp.tile([128, 512], F32, tag="pt", name="pt")
                            nc.scalar.activation(
                                out=pt[:],
                                in_=st[:],
                                func=mybir.ActivationFunctionType.Exp,
                            )
                            lhsT_v = (
                                va_sb[:, p, c, 0:65]
                                if even
                                else va_sb[:, p, c, 64:192]
                            )
                            nc.tensor.matmul(
                                ctx_out,
                                lhsT=lhsT_v,
                                rhs=pt[:],
                                start=(c == 0),
                                stop=(c == nch - 1),
                            )
                        # normalization + bias, partition-aligned per parity
                        cs = 64 if even else 0
                        lo = 0 if even else 64
                        r = smp.tile([1, 512], F32, tag="r", name="r")
                        nc.vector.reciprocal(out=r[:], in_=ctx_ps[cs:cs + 1, :])
                        bc_ps = stp.tile([128, 512], F32, tag="st", name="bc_ps")
                        if even:
                            nc.tensor.matmul(
                                bc_ps[0:64, :],
                                lhsT=ones_sb[0:1, 0:64],
                                rhs=r[:],
                                start=True,
                                stop=True,
                            )
                        else:
                            nc.tensor.matmul(
                                bc_ps[:],
                                lhsT=oneshalf_sb[0:1, :],
                                rhs=r[:],
                                start=True,
                                stop=True,
                            )
                        bc_sb = smp.tile([128, 512], F32, tag="bc", name="bc_sb")
                        nc.vector.tensor_copy(
                            out=bc_sb[lo:lo + 64, :], in_=bc_ps[lo:lo + 64, :]
                        )
                        tn = smp.tile([128, 512], F32, tag="tn", name="tn")
                        nc.vector.tensor_mul(
                            out=tn[lo:lo + 64, :],
                            in0=ctx_ps[lo:lo + 64, :],
                            in1=bc_sb[lo:lo + 64, :],
                        )
                        nc.vector.tensor_scalar_add(
                            out=ctxcat_sb[lo:lo + 64, p, qb * 512:(qb + 1) * 512],
                            in0=tn[lo:lo + 64, :],
                            scalar1=bv_sb[lo:lo + 64, h:h + 1],
                        )

                    # output projection for this query block's rows
                    for s16 in range(qb * 4, (qb + 1) * 4):
                        for do in range(2):
                            op = mmp.tile([128, 512], F32, tag="mm", name="op")
                            nc.tensor.matmul(
                                op[:],
                                lhsT=ctxcat_sb[:, 0, s16 * 128:(s16 + 1) * 128],
                                rhs=wo_sb[:, 0, do * 512:(do + 1) * 512],
                                start=True,
                                stop=False,
                            )
                            nc.tensor.matmul(
                                op[:],
                                lhsT=ctxcat_sb[:, 1, s16 * 128:(s16 + 1) * 128],
                                rhs=wo_sb[:, 1, do * 512:(do + 1) * 512],
                                start=False,
                                stop=False,
                            )
                            nc.tensor.matmul(
                                op[:],
                                lhsT=ones_sb[0:1, :],
                                rhs=bo_sb[0:1, do * 512:(do + 1) * 512],
                                start=False,
                                stop=True,
                            )
                            ot = outp.tile([128, 512], F32, tag="ot", name="ot")
                            nc.vector.tensor_copy(out=ot[:], in_=op[:])
                            nc.sync.dma_start(
                                out=out[s16 * 128:(s16 + 1) * 128,
                                        do * 512:(do + 1) * 512],
                                in_=ot[:],
                            )
    if not nc.is_finalized():
        nc.finalize()
    return nc


def _prep_inputs(embeddings, Wq, bq, Wk, bk, Wv, bv, Wo, bo):
    embeddings = np.asarray(embeddings, np.float32)
    Wq, bq = np.asarray(Wq, np.float32), np.asarray(bq, np.float32)
    Wk, bk = np.asarray(Wk, np.float32), np.asarray(bk, np.float32)
    Wv, bv = np.asarray(Wv, np.float32), np.asarray(bv, np.float32)
    Wo, bo = np.asarray(Wo, np.float32), np.asarray(bo, np.float32)

    p_idx = np.arange(128)
    mneg = np.zeros((128, 4, 512), np.float32)
    for i in range(4):
        f = np.arange(512)[None, :]
        mneg[:, i, :] = np.where(f >= p_idx[:, None] + 128 * i, 0.0, -1e9)
    ident = np.eye(128, dtype=np.float32)
    ones = np.ones((1, 128), np.float32)
    oneshalf = np.concatenate(
        [np.zeros((1, 64), np.float32), np.ones((1, 64), np.float32)], axis=1
    )

    in_maps = []
    for c in range(NCORES):
        b, g = c // 4, c % 4
        hs = HPC * g
        xt = np.ascontiguousarray(embeddings[b].T)
        wq2 = np.stack(
            [np.concatenate([Wq[hs + 2 * p], Wq[hs + 2 * p + 1]], axis=1)
             for p in range(NPAIR)]
        )
        wk2 = np.stack(
            [np.concatenate([Wk[hs + 2 * p], Wk[hs + 2 * p + 1]], axis=1)
             for p in range(NPAIR)]
        )
        wv4 = np.concatenate([Wv[hs + h] for h in range(HPC)], axis=1)
        wo4 = np.ascontiguousarray(Wo[hs * DK:(hs + HPC) * DK, :])
        bq2 = np.stack(
            [np.concatenate([bq[hs + 2 * p], bq[hs + 2 * p + 1]]) / 8.0
             for p in range(NPAIR)], axis=1
        )
        bk2 = np.stack(
            [np.concatenate([bk[hs + 2 * p], bk[hs + 2 * p + 1]])
             for p in range(NPAIR)], axis=1
        )
        bv4 = np.stack(
            [np.tile(bv[hs + h], 2) for h in range(HPC)], axis=1
        )
        bo1 = (bo if g == 0 else np.zeros_like(bo)).reshape(1, D)
        in_maps.append({
            "xt": np.ascontiguousarray(xt),
            "wq": np.ascontiguousarray(wq2),
            "wk": np.ascontiguousarray(wk2),
            "wv": np.ascontiguousarray(wv4),
            "wo": wo4,
            "bq": np.ascontiguousarray(bq2),
            "bk": np.ascontiguousarray(bk2),
            "bv": np.ascontiguousarray(bv4),
            "bo": np.ascontiguousarray(bo1),
            "mneg": mneg,
            "ident": ident,
            "ones": ones,
            "oneshalf": oneshalf,
        })
    return in_maps


def kernel(embeddings, Wq, bq, Wk, bk, Wv, bv, Wo, bo, _trace=False, _trace_kw=None):
    if "nc" not in _CACHE:
        _CACHE["nc"] = _build_bass()
    nc = _CACHE["nc"]
    in_maps = _prep_inputs(embeddings, Wq, bq, Wk, bk, Wv, bv, Wo, bo)
    kw = dict(_trace_kw or {})
    res = run_bass_kernel_spmd(
        nc, in_maps, core_ids=list(range(NCORES)), trace=_trace, **kw
    )
    _CACHE["last_result"] = res
    out = np.empty((B, S, D), np.float32)
    for b in range(B):
        acc = np.array(res.results[4 * b]["out"], np.float32, copy=True)
        for g in range(1, 4):
            acc += np.asarray(res.results[4 * b + g]["out"], np.float32)
        out[b] = acc
    return out
